# revision 2
# baseline (speedup 1.0000x reference)
"""Pointer-network additive-attention kernel for 8 Trainium2 NeuronCores.

Reference computation (B=4, DEC=256, ENC=1024, H=256):
    enc_t = x_encoder @ w1.T            # (B, ENC, H)
    dec_t = x_decoder @ w2.T            # (B, DEC, H)
    scores[b,d,e] = sum_h v[h] * tanh(dec_t[b,d,h] + enc_t[b,e,h])
    out = log_softmax(scores, axis=-1)  # (B, DEC, ENC)

Sharding: 8 cores = (batch b, decoder half dh).  Each core owns 128 decoder
rows of one batch and the full encoder axis, so log_softmax is core-local
(no collectives).

Per-core algorithm (h on partitions, 2 chunks of 128):
  - enc_T[hchunk] = (H x ENC) tile of enc_t transposed  (PE matmul, fp32)
  - dec_T[hchunk] = (H x 128d) tile of dec_t transposed (PE matmul, fp32)
  - for each d: T = tanh(enc_T + dec_col[d]) fused on ScalarE (bias add),
    output bf16; scores^T column via PE matmul with T as stationary and
    v as moving -> PSUM (e x d) accumulated over the two h chunks.
  - PE-transpose scores^T -> (d x e), log_softmax on DVE/ScalarE.
"""

import numpy as np
from contextlib import ExitStack

import concourse.bass as bass
import concourse.tile as tile
from concourse import bacc, masks, mybir
from concourse.bass_utils import run_bass_kernel_spmd

AF = mybir.ActivationFunctionType
ALU = mybir.AluOpType
AX = mybir.AxisListType
F32 = mybir.dt.float32
BF16 = mybir.dt.bfloat16

B, DEC, ENC, H = 4, 256, 1024, 256
NCORES = 8
DPC = 128          # decoder rows per core
HC = 128           # h chunk (partition dim)
NH = H // HC       # 2 h chunks
NE = ENC // 128    # 8 encoder chunks of 128


def build_nc(repeat: int = 1):
    nc = bacc.Bacc("TRN2", target_bir_lowering=False, debug=False,
                   num_devices=NCORES)

    xeT = nc.dram_tensor("xeT", (H, ENC), F32, kind="ExternalInput")
    xdT = nc.dram_tensor("xdT", (H, DPC), F32, kind="ExternalInput")
    w1T = nc.dram_tensor("w1T", (H, H), F32, kind="ExternalInput")
    w2T = nc.dram_tensor("w2T", (H, H), F32, kind="ExternalInput")
    vcol = nc.dram_tensor("vcol", (H, 1), F32, kind="ExternalInput")
    out = nc.dram_tensor("out", (DPC, ENC), F32, kind="ExternalOutput")

    with tile.TileContext(nc) as tc, ExitStack() as ctx:
        const = ctx.enter_context(tc.tile_pool(name="const", bufs=1))
        work = ctx.enter_context(tc.tile_pool(name="work", bufs=2))
        stat = ctx.enter_context(tc.tile_pool(name="stat", bufs=2))
        tpool = ctx.enter_context(tc.tile_pool(name="tanh", bufs=4))
        pbig = ctx.enter_context(
            tc.tile_pool(name="pbig", bufs=3, space=bass.MemorySpace.PSUM))
        psmall = ctx.enter_context(
            tc.tile_pool(name="psmall", bufs=2, space=bass.MemorySpace.PSUM))

        ident = const.tile([128, 128], F32, tag="ident")
        masks.make_identity(nc, ident[:])

        for _rep in range(repeat):
            # ---- stage 0: loads ----
            xe_sb = [work.tile([HC, ENC], F32, tag=f"xe{c}", name=f"xe{c}") for c in range(NH)]
            for c in range(NH):
                nc.sync.dma_start(xe_sb[c][:], xeT[c * HC:(c + 1) * HC, :])
            w1_sb = [[work.tile([HC, HC], F32, tag=f"w1_{hc}{oc}", name=f"w1_{hc}{oc}")
                      for oc in range(NH)] for hc in range(NH)]
            w2_sb = [[work.tile([HC, HC], F32, tag=f"w2_{hc}{oc}", name=f"w2_{hc}{oc}")
                      for oc in range(NH)] for hc in range(NH)]
            for hc in range(NH):
                for oc in range(NH):
                    nc.sync.dma_start(
                        w1_sb[hc][oc][:],
                        w1T[hc * HC:(hc + 1) * HC, oc * HC:(oc + 1) * HC])
                    nc.sync.dma_start(
                        w2_sb[hc][oc][:],
                        w2T[hc * HC:(hc + 1) * HC, oc * HC:(oc + 1) * HC])
            xd_sb = [work.tile([HC, DPC], F32, tag=f"xd{c}", name=f"xd{c}") for c in range(NH)]
            for c in range(NH):
                nc.sync.dma_start(xd_sb[c][:], xdT[c * HC:(c + 1) * HC, :])
            vf_sb = [work.tile([HC, 1], F32, tag=f"vf{c}", name=f"vf{c}") for c in range(NH)]
            vb_sb = [work.tile([HC, 1], BF16, tag=f"vb{c}", name=f"vb{c}") for c in range(NH)]
            for c in range(NH):
                nc.sync.dma_start(vf_sb[c][:], vcol[c * HC:(c + 1) * HC, :])
                nc.vector.tensor_copy(vb_sb[c][:], vf_sb[c][:])

            # ---- stage 1: enc_T / dec_T projections (fp32 matmuls) ----
            encT = [work.tile([HC, ENC], F32, tag=f"encT{c}", name=f"encT{c}") for c in range(NH)]
            decT = [work.tile([HC, DPC], F32, tag=f"decT{c}", name=f"decT{c}") for c in range(NH)]
            for oc in range(NH):
                pe = pbig.tile([128, ENC], F32, tag="pbig")
                for eh in range(ENC // 512):
                    for hc in range(NH):
                        nc.tensor.matmul(
                            pe[:, eh * 512:(eh + 1) * 512],
                            w1_sb[hc][oc][:],
                            xe_sb[hc][:, eh * 512:(eh + 1) * 512],
                            start=(hc == 0), stop=(hc == NH - 1))
                nc.vector.tensor_copy(encT[oc][:], pe[:])
            for oc in range(NH):
                pd = psmall.tile([128, DPC], F32, tag="psmall")
                for hc in range(NH):
                    nc.tensor.matmul(pd[:], w2_sb[hc][oc][:], xd_sb[hc][:],
                                     start=(hc == 0), stop=(hc == NH - 1))
                nc.vector.tensor_copy(decT[oc][:], pd[:])

            # ---- stage 2: tanh + v-dot -> scores^T in PSUM ----
            # sc_ps[:, ec*128 + d] = scores^T column: e in chunk ec (partition), d
            sc_ps = pbig.tile([128, ENC], F32, tag="pbig")
            for d in range(DPC):
                tt = [tpool.tile([HC, ENC], BF16, tag=f"T{c}", name=f"T{c}") for c in range(NH)]
                for c in range(NH):
                    nc.scalar.activation(tt[c][:], encT[c][:], AF.Tanh,
                                         bias=decT[c][:, d:d + 1])
                for ec in range(NE):
                    col = ec * 128 + d
                    for c in range(NH):
                        nc.tensor.matmul(
                            sc_ps[:, col:col + 1],
                            tt[c][:, ec * 128:(ec + 1) * 128],
                            vb_sb[c][:],
                            start=(c == 0), stop=(c == NH - 1))

            # ---- stage 3: transpose + log_softmax ----
            scT_sb = work.tile([128, ENC], F32, tag="scT")
            nc.vector.tensor_copy(scT_sb[:], sc_ps[:])
            sc2 = pbig.tile([128, ENC], F32, tag="pbig")
            for ec in range(NE):
                nc.tensor.transpose(sc2[:, ec * 128:(ec + 1) * 128],
                                    scT_sb[:, ec * 128:(ec + 1) * 128],
                                    ident[:])
            mneg = stat.tile([128, 1], F32, tag="mneg")
            nc.vector.reduce_max(mneg[:], sc2[:], axis=AX.X, negate=True)
            p_sb = work.tile([128, ENC], F32, tag="p_sb")
            ssum = stat.tile([128, 1], F32, tag="ssum")
            nc.scalar.activation(p_sb[:], sc2[:], AF.Exp,
                                 bias=mneg[:, 0:1], accum_out=ssum[:])
            lse = stat.tile([128, 1], F32, tag="lse")
            nc.scalar.activation(lse[:], ssum[:], AF.Ln)
            mls = stat.tile([128, 1], F32, tag="mls")
            nc.vector.tensor_sub(mls[:], mneg[:], lse[:])
            out_sb = work.tile([128, ENC], F32, tag="out_sb")
            nc.vector.tensor_scalar_add(out_sb[:], sc2[:], mls[:, 0:1])
            nc.sync.dma_start(out[:], out_sb[:])

    nc.compile()
    return nc


def make_in_maps(x_decoder, x_encoder, w1, w2, v):
    w1T = np.ascontiguousarray(w1.T)
    w2T = np.ascontiguousarray(w2.T)
    vc = np.ascontiguousarray(v.reshape(H, 1))
    in_maps = []
    for c in range(NCORES):
        b, dh = divmod(c, 2)
        in_maps.append({
            "xeT": np.ascontiguousarray(x_encoder[b].T),
            "xdT": np.ascontiguousarray(x_decoder[b, dh * DPC:(dh + 1) * DPC, :].T),
            "w1T": w1T,
            "w2T": w2T,
            "vcol": vc,
        })
    return in_maps


_NC_CACHE = {}


def _get_nc(repeat: int = 1):
    if repeat not in _NC_CACHE:
        _NC_CACHE[repeat] = build_nc(repeat)
    return _NC_CACHE[repeat]


def kernel(x_decoder, x_encoder, w1, w2, v):
    x_decoder = np.asarray(x_decoder, dtype=np.float32)
    x_encoder = np.asarray(x_encoder, dtype=np.float32)
    w1 = np.asarray(w1, dtype=np.float32)
    w2 = np.asarray(w2, dtype=np.float32)
    v = np.asarray(v, dtype=np.float32)

    nc = _get_nc(1)
    in_maps = make_in_maps(x_decoder, x_encoder, w1, w2, v)
    res = run_bass_kernel_spmd(nc, in_maps, core_ids=list(range(NCORES)))
    out = np.empty((B, DEC, ENC), dtype=np.float32)
    for c in range(NCORES):
        b, dh = divmod(c, 2)
        out[b, dh * DPC:(dh + 1) * DPC, :] = res.results[c]["out"]
    return out
